# revision 11
# baseline (speedup 1.0000x reference)
"""MatchingNet model kernel for 8 Trainium2 NeuronCores.

Computation (reference semantics, N=4096, E=512, G=256, V=50000, R=1000):
  x  = embedding[input]          (N, E)
  ex = embedding[set_inputs]     (2, N, E)
  g_out = bidirectional 2-step LSTM over ex   (2, N, E)
  fh = lstm_f(x) + x             (N, E)          [single step, zero state]
  scores[b] = g_out[b] @ fh.T    (2, N, N)
  a = softmax(scores, axis=0)    -- softmax over b (size 2), pointwise in (n,m)
  r[b] = a[b] @ g_out[b]         (2, N, E)
  dot/nr/ng reductions over n -> cos (2, E) -> tiny tail -> softmax (R,)

Sharding: data-parallel over N. Core k owns rows [512k, 512k+512).
Attention trick: a[0] = sigmoid(D), a[1] = sigmoid(-D) with
D = (g0 - g1) @ fh.T, so only 3 N*N*E matmuls instead of 4.

fp8 (e4m3) is used for all heavy matmuls via DoubleRow perf mode (2x PE
rate, contraction pairs laid out as [p, 2, f]) and for both all-gather
payloads (fh keys + g values).  LSTM weights are pre-scaled x16 on the
host (e4m3 normal range) and unscaled in the gate activation; dg is
scaled x8 before the cast and unscaled in the sigmoid.  All reductions
(sg/sr/dot) run in fp32 from fp32 copies / PSUM.  Collective bounce
buffers are declared flat (1 row) - collective cost scales with
descriptor rows, not just bytes.

Each core reduces over its n rows to partials dot/sr/sg (2, E); the
host sums partials and runs the O(R*E) tail (cosine normalize,
projection collapse, final softmax).
"""

import os
import sys

import numpy as np

for _p in ("/opt/trn_rl_repo", os.path.expanduser("~/.axon_site/_ro/trn_rl_repo")):
    if os.path.isdir(_p) and _p not in sys.path:
        sys.path.insert(0, _p)

import concourse.bacc as bacc
import concourse.bass as bass
import concourse.mybir as mybir
import concourse.tile as tile
from concourse import bass_utils
from concourse.masks import make_identity

N, E, G, V, R = 4096, 512, 256, 50000, 1000
NCORES = 8
NL = N // NCORES  # 512 rows per core
P = 128
NE = E // P   # 4 e-chunks
NH = G // P   # 2 hidden chunks for the g-LSTM
NMB = N // P  # 32 m-blocks
EPS = 1e-8
WSCALE = 16.0
DGSCALE = 8.0

F32 = mybir.dt.float32
F32R = mybir.dt.float32r
F8 = mybir.dt.float8e4
I32 = mybir.dt.int32
AF = mybir.ActivationFunctionType
ALU = mybir.AluOpType
DR = mybir.MatmulPerfMode.DoubleRow


def _lstm_cell8(nc, pools, H, xT8, W_sb, U_sb, h8, bias_sb, h_out, c_out,
                cprevT=None, h_out8=None):
    """One LSTM cell, fp8 DoubleRow gates, transposed layout.

    xT8: (P, NE, NL) fp8; W_sb: (P, NE, 4H) fp8 (host-scaled x16);
    U_sb: (P, NH, 4H) fp8 or None; h8: (P, NH, NL) fp8 prev hidden.
    h_out/c_out: (P, H//P, NL) fp32 views. cprevT None => forget gate
    skipped (zero initial state) and c = sig(i)*tanh(g).
    h_out8: optional fp8 copy of h (for a later U-matmul).
    """
    pg, gp, tp = pools["pg"], pools["gates"], pools["tmp"]
    hc = H // P
    nj = 4 * H // P
    gb = [gp.tile([P, hc, NL], F32, tag=f"gate{g}", bufs=1, name=f"gb{g}")
          for g in range(4)]
    if c_out is None:
        assert cprevT is None
        c_out = gb[1]  # forget-gate buffer is unused for zero-state cells
    for jc in range(nj):
        g = jc // hc
        if cprevT is None and g == 1:
            continue  # forget gate unused with zero initial state
        ps = pg.tile([P, NL], F32, tag="pg", bufs=4, name="ps_gate")
        js = slice(jc * P, (jc + 1) * P)
        for i in range(NE // 2):
            ks = slice(2 * i, 2 * i + 2)
            nc.tensor.matmul(
                ps[:], W_sb[:, ks, js], xT8[:, ks, :],
                start=(i == 0), stop=(U_sb is None and i == NE // 2 - 1),
                perf_mode=DR)
        if U_sb is not None:
            nc.tensor.matmul(
                ps[:], U_sb[:, 0:NH, js], h8[:, 0:NH, :],
                start=False, stop=True, perf_mode=DR)
        func = AF.Tanh if g == 2 else AF.Sigmoid
        nc.scalar.activation(
            out=gb[g][:, jc % hc, :], in_=ps[:], func=func,
            bias=bias_sb[:, jc:jc + 1], scale=1.0 / WSCALE)
    for s in range(hc):
        i_, g_, o_ = gb[0][:, s, :], gb[2][:, s, :], gb[3][:, s, :]
        if cprevT is None:
            nc.vector.tensor_mul(c_out[:, s, :], i_, g_)
        else:
            f_ = gb[1][:, s, :]
            ig = tp.tile([P, NL], F32, tag="ig", bufs=2, name="ig")
            nc.vector.tensor_mul(ig[:], i_, g_)
            nc.vector.tensor_mul(c_out[:, s, :], f_, cprevT[:, s, :])
            nc.vector.tensor_add(c_out[:, s, :], c_out[:, s, :], ig[:])
        tc_ = tp.tile([P, NL], F32, tag="tanhc", bufs=2, name="tanhc")
        nc.scalar.activation(out=tc_[:], in_=c_out[:, s, :], func=AF.Tanh)
        nc.vector.tensor_mul(h_out[:, s, :], o_, tc_[:])
        if h_out8 is not None:
            nc.vector.tensor_copy(out=h_out8[:, s, :],
                                  in_=h_out[:, s, :].bitcast(F32))


def _gather_T(nc, pools, emb, idx_dram, identr, dst8, dst32=None):
    """Gather NL embedding rows, transpose, cast to fp8 (and opt. fp32)."""
    ip, rp, ptp = pools["idx"], pools["raw"], pools["pt"]
    for t in range(NL // P):
        idx_t = ip.tile([P, 1], I32, tag="idx", bufs=4, name="idx_t")
        nc.sync.dma_start(out=idx_t[:], in_=idx_dram[t * P:(t + 1) * P, :])
        raw = rp.tile([P, E], F32R, tag="raw", bufs=4, name="raw")
        nc.gpsimd.indirect_dma_start(
            out=raw[:], out_offset=None, in_=emb[:],
            in_offset=bass.IndirectOffsetOnAxis(ap=idx_t[:, :1], axis=0))
        ptile = ptp.tile([P, NE, P], F32R, tag="pt", bufs=3, name="ptile")
        for et in range(NE):
            nc.tensor.transpose(
                out=ptile[:, et, :], in_=raw[:, et * P:(et + 1) * P],
                identity=identr[:])
        sl = slice(t * P, (t + 1) * P)
        nc.vector.tensor_copy(out=dst8[:, :, sl],
                              in_=ptile[:].bitcast(F32))
        if dst32 is not None:
            nc.scalar.activation(out=dst32[:, :, sl],
                                 in_=ptile[:].bitcast(F32), func=AF.Copy)


def build_program():
    nc = bacc.Bacc("TRN2", target_bir_lowering=False, debug=False,
                   enable_asserts=False, num_devices=NCORES)
    dram = lambda name, shape, dt=F32, kind="ExternalInput": \
        nc.dram_tensor(name, shape, dt, kind=kind).ap()

    emb = dram("emb", [V, E])
    idx_x = dram("idx_x", [NL, 1], I32)
    idx_e0 = dram("idx_e0", [NL, 1], I32)
    idx_e1 = dram("idx_e1", [NL, 1], I32)
    # weights pre-laid-out on host as fp8 lhsT tiles [p, kt, j], x16
    wgf = dram("wgf", [P, NE, 4 * G], F8)
    wgr = dram("wgr", [P, NE, 4 * G], F8)
    ugf = dram("ugf", [P, NH, 4 * G], F8)
    ugr = dram("ugr", [P, NH, 4 * G], F8)
    wf = dram("wf", [P, NE, 4 * E], F8)
    bgf = dram("bgf", [P, 8])
    bgr = dram("bgr", [P, 8])
    bf = dram("bf", [P, 16])
    out_dot = dram("out_dot", [2, E], kind="ExternalOutput")
    out_sr = dram("out_sr", [2, E], kind="ExternalOutput")
    out_sg = dram("out_sg", [2, E], kind="ExternalOutput")

    with tile.TileContext(nc) as tc:
        _emit(tc, locals())
    nc.compile()
    return nc


def _emit(tc, T):
    nc = tc.nc
    rg = [list(range(NCORES))]
    from contextlib import ExitStack
    ctx = ExitStack()
    with ctx:
        glob = ctx.enter_context(tc.tile_pool(name="glob", bufs=1))
        dramp = ctx.enter_context(tc.tile_pool(name="dramp", bufs=1, space="DRAM"))

        # tiny warmup collective: absorbs the device barrier + first-
        # collective ncfw warmup (~45us) while local compute proceeds.
        # No producers (payload is scratch) so the trigger is ready at t=0.
        warm_src = dramp.tile([1, 256], F8)
        warm_dst = dramp.tile([NCORES, 256], F8, addr_space="Shared")
        with tc.high_priority():
            nc.gpsimd.collective_compute(
                "AllGather", ALU.bypass, replica_groups=rg,
                ins=[warm_src[:].opt()], outs=[warm_dst[:].opt()])

        ident = glob.tile([P, P], F32)
        make_identity(nc, ident)
        identr = glob.tile([P, P], F32R)
        nc.vector.tensor_copy(out=identr[:], in_=ident[:])

        # flat collective bounce buffers (fp8): descriptor-row count, not
        # bytes, dominates collective cost - declare 1 row, view for DMA
        ag1_src = dramp.tile([1, E * NL], F8)
        ag1_dst = dramp.tile([NCORES, E * NL], F8, addr_space="Shared")
        ag2_src = dramp.tile([1, 2 * NL * E], F8)
        ag2_dst = dramp.tile([NCORES, 2 * NL * E], F8, addr_space="Shared")
        ag1w = ag1_src.rearrange("a (e n) -> (a e) n", n=NL)       # [E, NL]
        ag2w = ag2_src.rearrange("a (m e) -> (a m) e", e=E)        # [2NL, E]

        # long-lived activations
        xT8 = glob.tile([P, NE, NL], F8)
        xT = glob.tile([P, NE, NL], F32)
        e0T8 = glob.tile([P, NE, NL], F8)
        e1T8 = glob.tile([P, NE, NL], F8)
        g0T = glob.tile([P, NE, NL], F32R)
        g1T = glob.tile([P, NE, NL], F32R)
        h08 = glob.tile([P, NH, NL], F8)   # hf0 fp8 (U input of fwd step 2)
        h18 = glob.tile([P, NH, NL], F8)   # hr1 fp8 (U input of rev step 2)
        dgT8 = glob.tile([P, NE, NL], F8)
        A0T = glob.tile([P, NMB, NL], F8)
        A1T = glob.tile([P, NMB, NL], F8)

        with tc.tile_pool(name="wpool", bufs=1) as wp, \
             tc.tile_pool(name="acts", bufs=1) as ap_, \
             tc.tile_pool(name="gates", bufs=1) as gp, \
             tc.tile_pool(name="tmp", bufs=1) as tp, \
             tc.tile_pool(name="idx", bufs=1) as ip, \
             tc.tile_pool(name="raw", bufs=1) as rp, \
             tc.tile_pool(name="pg", bufs=1, space="PSUM") as pgp, \
             tc.tile_pool(name="pt", bufs=1, space="PSUM") as ptp:
            pools = {"pg": pgp, "gates": gp, "tmp": tp, "idx": ip,
                     "raw": rp, "pt": ptp}

            # ---- phase A: x gather -> f-cell -> fh -> AG1 (early) ----
            _gather_T(nc, pools, T["emb"], T["idx_x"], identr, xT8, xT)
            wf_sb = wp.tile([P, NE, 4 * E], F8)
            nc.sync.dma_start(out=wf_sb[:], in_=T["wf"][:])
            bf_sb = wp.tile([P, 16], F32)
            nc.sync.dma_start(out=bf_sb[:], in_=T["bf"][:])

            fhT = ap_.tile([P, NE, NL], F32, name="fhT")
            _lstm_cell8(nc, pools, E, xT8, wf_sb, None, None, bf_sb, fhT,
                        None)
            fhT8 = ap_.tile([P, NE, NL], F8, name="fhT8")
            for et in range(NE):
                nc.vector.tensor_add(fhT[:, et, :], fhT[:, et, :],
                                     xT[:, et, :])
                nc.vector.tensor_copy(out=fhT8[:, et, :], in_=fhT[:, et, :])
                nc.sync.dma_start(
                    out=ag1w[et * P:(et + 1) * P, :], in_=fhT8[:, et, :])
            nc.gpsimd.collective_compute(
                "AllGather", ALU.bypass, replica_groups=rg,
                ins=[ag1_src[:].opt()], outs=[ag1_dst[:].opt()])

            # ---- phase B: e gathers -> g cells -> transposes -> AG2 ----
            _gather_T(nc, pools, T["emb"], T["idx_e0"], identr, e0T8)
            _gather_T(nc, pools, T["emb"], T["idx_e1"], identr, e1T8)
            w_sb = {}
            for nm in ("wgf", "wgr", "ugf", "ugr"):
                shp = [P, NE, 4 * G] if nm[0] == "w" else [P, NH, 4 * G]
                w_sb[nm] = wp.tile(shp, F8, name=nm + "_sb")
                nc.sync.dma_start(out=w_sb[nm][:], in_=T[nm][:])
            for nm in ("bgf", "bgr"):
                w_sb[nm] = wp.tile([P, 8], F32, name=nm + "_sb")
                nc.sync.dma_start(out=w_sb[nm][:], in_=T[nm][:])

            cfT = ap_.tile([P, NH, NL], F32, name="cfT")
            crT = ap_.tile([P, NH, NL], F32, name="crT")
            c2T = ap_.tile([P, NH, NL], F32, name="c2T")
            c3T = ap_.tile([P, NH, NL], F32, name="c3T")
            hf0 = g0T[:, 0:NH, :]
            hf1 = g1T[:, 0:NH, :]
            hr1 = g1T[:, NH:NE, :]
            hr0 = g0T[:, NH:NE, :]
            _lstm_cell8(nc, pools, G, e0T8, w_sb["wgf"], None, None,
                        w_sb["bgf"], hf0, cfT, h_out8=h08)
            _lstm_cell8(nc, pools, G, e1T8, w_sb["wgr"], None, None,
                        w_sb["bgr"], hr1, crT, h_out8=h18)
            _lstm_cell8(nc, pools, G, e1T8, w_sb["wgf"], w_sb["ugf"], h08,
                        w_sb["bgf"], hf1, c2T, cprevT=cfT)
            _lstm_cell8(nc, pools, G, e0T8, w_sb["wgr"], w_sb["ugr"], h18,
                        w_sb["bgr"], hr0, c3T, cprevT=crT)

            # dg = g0 - g1, scaled x8 into fp8 (unscaled in the D sigmoid)
            for et in range(NE):
                dgf = tp.tile([P, NL], F32, tag="dgf", bufs=2, name="dgf")
                nc.vector.tensor_sub(dgf[:], g0T[:, et, :].bitcast(F32),
                                     g1T[:, et, :].bitcast(F32))
                nc.vector.tensor_scalar_mul(
                    out=dgT8[:, et, :], in0=dgf[:], scalar1=DGSCALE)

            # transpose g0/g1 into ag2_src (fp8) and fire AG2.
            # high_priority: the stile DMAs and the trigger must win queue
            # slots over AG1-gated loads and reduction outputs.
            with tc.high_priority():
                for srcT, row0 in ((g0T, 0), (g1T, NL)):
                    for nt in range(NL // P):
                        ptile = ptp.tile([P, NE, P], F32R, tag="pt", bufs=3,
                                         name="ptg")
                        for et in range(NE):
                            nc.tensor.transpose(
                                out=ptile[:, et, :],
                                in_=srcT[:, et, nt * P:(nt + 1) * P],
                                identity=identr[:])
                        stile = tp.tile([P, E], F8, tag="tps", bufs=8,
                                        name="stile")
                        nc.vector.tensor_copy(out=stile[:],
                                              in_=ptile[:].bitcast(F32))
                        nc.sync.dma_start(
                            out=ag2w[row0 + nt * P:row0 + (nt + 1) * P, :],
                            in_=stile[:])
                nc.gpsimd.collective_compute(
                    "AllGather", ALU.bypass, replica_groups=rg,
                    ins=[ag2_src[:].opt()], outs=[ag2_dst[:].opt()])


        # ---- phase C: D.T blocks; A0 = sig(D/8), A1 = 1 - A0 (fp8) ----
        with tc.tile_pool(name="fhk", bufs=1) as fkp, \
             tc.tile_pool(name="gS", bufs=1) as gsp, \
             tc.tile_pool(name="fin", bufs=1) as fin, \
             tc.tile_pool(name="pd", bufs=1, space="PSUM") as pdp, \
             tc.tile_pool(name="pr", bufs=1, space="PSUM") as prp:
            for k in range(NCORES):
                fhk = fkp.tile([P, NE, NL], F8, tag="fhk", bufs=4, name="fhk")
                nc.sync.dma_start(
                    out=fhk[:],
                    in_=ag1_dst[k:k + 1, :].rearrange(
                        "k (et p n) -> (k p) et n", p=P, n=NL))
                for c in range(NL // P):
                    mb = k * (NL // P) + c
                    pd = pdp.tile([P, NL], F32, tag="pd", bufs=4, name="pd")
                    cs = slice(c * P, (c + 1) * P)
                    for i in range(NE // 2):
                        ks = slice(2 * i, 2 * i + 2)
                        nc.tensor.matmul(
                            pd[:], fhk[:, ks, cs], dgT8[:, ks, :],
                            start=(i == 0), stop=(i == NE // 2 - 1),
                            perf_mode=DR)
                    nc.scalar.activation(
                        out=A0T[:, mb, :], in_=pd[:], func=AF.Sigmoid,
                        scale=1.0 / DGSCALE)
                for c2 in range(0, NL // P, 2):
                    mb = k * (NL // P) + c2
                    nc.vector.tensor_scalar(
                        out=A1T[:, mb:mb + 2, :], in0=A0T[:, mb:mb + 2, :],
                        scalar1=-1.0, scalar2=1.0, op0=ALU.mult, op1=ALU.add)

            # ---- phase D: r via fp8 DoubleRow, g cached in SBUF ----
            g0S = gsp.tile([P, NMB, E], F8, name="g0S")
            g1S = gsp.tile([P, NMB, E], F8, name="g1S")
            nb = NL // P
            for gS, off in ((g0S, 0), (g1S, NL * E)):
                for k in range(NCORES):
                    nc.sync.dma_start(
                        out=gS[:, k * nb:(k + 1) * nb, :],
                        in_=ag2_dst[k:k + 1, off:off + NL * E].rearrange(
                            "k (c p e) -> (k p) c e", p=P, e=E))

            # sg_b = sum_n g_b^2 (local, fp32) - emitted late so its
            # scalar ops and tiny DMAs sit behind the D-phase in queue order
            for b, gT in ((0, g0T), (1, g1T)):
                for et in range(NE):
                    scr3 = fin.tile([P, NL], F32, tag="scr3", bufs=4,
                                    name="scr3")
                    asg = fin.tile([P, 1], F32, tag="asg", bufs=8, name="asg")
                    nc.scalar.activation(out=scr3[:],
                                         in_=gT[:, et, :].bitcast(F32),
                                         func=AF.Square, accum_out=asg[:])
                    nc.sync.dma_start(out=T["out_sg"][b, et * P:(et + 1) * P],
                                      in_=asg[:])

            for et in range(NE):
                es = slice(et * P, (et + 1) * P)
                r0 = prp.tile([P, NL], F32, tag="r0", bufs=2, name="r0")
                r1 = prp.tile([P, NL], F32, tag="r1", bufs=2, name="r1")
                for i in range(NMB // 2):
                    ms = slice(2 * i, 2 * i + 2)
                    nc.tensor.matmul(
                        r0[:], g0S[:, ms, es], A0T[:, ms, :],
                        start=(i == 0), stop=(i == NMB // 2 - 1),
                        perf_mode=DR)
                    nc.tensor.matmul(
                        r1[:], g1S[:, ms, es], A1T[:, ms, :],
                        start=(i == 0), stop=(i == NMB // 2 - 1),
                        perf_mode=DR)
                # sr and dot reductions for this e-chunk (fp32)
                for b, (rr, gT) in enumerate(((r0, g0T), (r1, g1T))):
                    scr2 = fin.tile([P, NL], F32, tag="scr2", bufs=4,
                                    name="scr2")
                    asr = fin.tile([P, 1], F32, tag="asr", bufs=4,
                                   name="asr")
                    nc.scalar.activation(out=scr2[:], in_=rr[:],
                                         func=AF.Square, accum_out=asr[:])
                    nc.sync.dma_start(
                        out=T["out_sr"][b, et * P:(et + 1) * P], in_=asr[:])
                    scr = fin.tile([P, NL], F32, tag="scr", bufs=4,
                                   name="scr")
                    adot = fin.tile([P, 1], F32, tag="adot", bufs=4,
                                    name="adot")
                    nc.vector.tensor_mul(scr[:], rr[:],
                                         gT[:, et, :].bitcast(F32))
                    nc.vector.reduce_sum(out=adot[:], in_=scr[:],
                                         axis=mybir.AxisListType.X)
                    nc.sync.dma_start(
                        out=T["out_dot"][b, et * P:(et + 1) * P],
                        in_=adot[:])


_PROGRAM = None


def _get_program():
    global _PROGRAM
    if _PROGRAM is None:
        _PROGRAM = build_program()
    return _PROGRAM


def _prep_w8(w, scale=WSCALE):
    """(4H, E_in) torch-layout weight -> fp8 lhsT tiles [p, kt, 4H], scaled."""
    wt = np.asarray(w, np.float32).T * scale  # (E_in, 4H)
    e_in, fourh = wt.shape
    arr = np.ascontiguousarray(
        wt.reshape(e_in // P, P, fourh).transpose(1, 0, 2))
    arr = np.clip(arr, -240.0, 240.0)
    return arr.astype(mybir.dt.np(F8))


def _prep_b(b1, b2):
    s = (np.asarray(b1, np.float32) + np.asarray(b2, np.float32))
    return np.ascontiguousarray(s.reshape(-1, P).T)


def run_device(inputs, trace=False):
    """Shard inputs, run the 8-core SPMD program, return bass results."""
    nc = _get_program()
    emb = np.ascontiguousarray(np.asarray(inputs["embedding"], np.float32))
    iq = np.asarray(inputs["input"]).astype(np.int32).reshape(N, 1)
    ie = np.asarray(inputs["set_inputs"]).astype(np.int32)
    shared = {
        "emb": emb,
        "wgf": _prep_w8(inputs["wih_gf"]), "wgr": _prep_w8(inputs["wih_gr"]),
        "ugf": _prep_w8(inputs["whh_gf"]), "ugr": _prep_w8(inputs["whh_gr"]),
        "wf": _prep_w8(inputs["wih_f"]),
        "bgf": _prep_b(inputs["bih_gf"], inputs["bhh_gf"]),
        "bgr": _prep_b(inputs["bih_gr"], inputs["bhh_gr"]),
        "bf": _prep_b(inputs["bih_f"], inputs["bhh_f"]),
    }
    in_maps = []
    for k in range(NCORES):
        sl = slice(k * NL, (k + 1) * NL)
        m = dict(shared)
        m["idx_x"] = np.ascontiguousarray(iq[sl])
        m["idx_e0"] = np.ascontiguousarray(ie[0, sl].reshape(NL, 1))
        m["idx_e1"] = np.ascontiguousarray(ie[1, sl].reshape(NL, 1))
        in_maps.append(m)
    res = bass_utils.run_bass_kernel_spmd(
        nc, in_maps, core_ids=list(range(NCORES)), trace=trace)
    return res


def kernel(**inputs):
    res = run_device(inputs)
    return host_tail(res, inputs)


def host_tail(res, inputs):
    dot = np.zeros((2, E), np.float64)
    sr = np.zeros((2, E), np.float64)
    sg = np.zeros((2, E), np.float64)
    for r in res.results:
        dot += r["out_dot"]
        sr += r["out_sr"]
        sg += r["out_sg"]
    nr = np.maximum(np.sqrt(sr), EPS)
    ng = np.maximum(np.sqrt(sg), EPS)
    cos = dot / (nr * ng)                        # (2, E)
    kern = cos / np.exp(cos).sum()
    w_out = np.asarray(inputs["w_out"], np.float64)
    b_out = np.asarray(inputs["b_out"], np.float64)
    k2 = kern @ w_out.T + b_out                  # (2, R)
    s = k2.sum(axis=1)                           # (2,)
    labels = np.asarray(inputs["set_labels"], np.float64)
    o = s[0] * labels[0] + s[1] * labels[1]      # (R,)
    o = np.exp(o - o.max())
    o /= o.sum()
    return o.astype(np.float32)


# revision 12
# speedup vs baseline: 1.0000x; 1.0000x over previous
"""MatchingNet model kernel for 8 Trainium2 NeuronCores.

Computation (reference semantics, N=4096, E=512, G=256, V=50000, R=1000):
  x  = embedding[input]          (N, E)
  ex = embedding[set_inputs]     (2, N, E)
  g_out = bidirectional 2-step LSTM over ex   (2, N, E)
  fh = lstm_f(x) + x             (N, E)          [single step, zero state]
  scores[b] = g_out[b] @ fh.T    (2, N, N)
  a = softmax(scores, axis=0)    -- softmax over b (size 2), pointwise in (n,m)
  r[b] = a[b] @ g_out[b]         (2, N, E)
  dot/nr/ng reductions over n -> cos (2, E) -> tiny tail -> softmax (R,)

Sharding: data-parallel over N. Core k owns rows [512k, 512k+512).
Attention trick: a[0] = sigmoid(D), a[1] = sigmoid(-D) with
D = (g0 - g1) @ fh.T, so only 3 N*N*E matmuls instead of 4.

fp8 (e4m3) is used for all heavy matmuls via DoubleRow perf mode (2x PE
rate, contraction pairs laid out as [p, 2, f]) and for both all-gather
payloads (fh keys + g values).  LSTM weights are pre-scaled x16 on the
host (e4m3 normal range) and unscaled in the gate activation; dg is
scaled x8 before the cast and unscaled in the sigmoid.  All reductions
(sg/sr/dot) run in fp32 from fp32 copies / PSUM.  Collective bounce
buffers are declared flat (1 row) - collective cost scales with
descriptor rows, not just bytes.

Each core reduces over its n rows to partials dot/sr/sg (2, E); the
host sums partials and runs the O(R*E) tail (cosine normalize,
projection collapse, final softmax).
"""

import os
import sys

import numpy as np

for _p in ("/opt/trn_rl_repo", os.path.expanduser("~/.axon_site/_ro/trn_rl_repo")):
    if os.path.isdir(_p) and _p not in sys.path:
        sys.path.insert(0, _p)

import concourse.bacc as bacc
import concourse.bass as bass
import concourse.mybir as mybir
import concourse.tile as tile
from concourse import bass_utils
from concourse.masks import make_identity

N, E, G, V, R = 4096, 512, 256, 50000, 1000
NCORES = 8
NL = N // NCORES  # 512 rows per core
P = 128
NE = E // P   # 4 e-chunks
NH = G // P   # 2 hidden chunks for the g-LSTM
NMB = N // P  # 32 m-blocks
EPS = 1e-8
WSCALE = 16.0
DGSCALE = 8.0

F32 = mybir.dt.float32
F32R = mybir.dt.float32r
F8 = mybir.dt.float8e4
I32 = mybir.dt.int32
AF = mybir.ActivationFunctionType
ALU = mybir.AluOpType
DR = mybir.MatmulPerfMode.DoubleRow


def _lstm_cell8(nc, pools, H, xT8, W_sb, U_sb, h8, bias_sb, h_out, c_out,
                cprevT=None, h_out8=None):
    """One LSTM cell, fp8 DoubleRow gates, transposed layout.

    xT8: (P, NE, NL) fp8; W_sb: (P, NE, 4H) fp8 (host-scaled x16);
    U_sb: (P, NH, 4H) fp8 or None; h8: (P, NH, NL) fp8 prev hidden.
    h_out/c_out: (P, H//P, NL) fp32 views. cprevT None => forget gate
    skipped (zero initial state) and c = sig(i)*tanh(g).
    h_out8: optional fp8 copy of h (for a later U-matmul).
    """
    pg, gp, tp = pools["pg"], pools["gates"], pools["tmp"]
    hc = H // P
    nj = 4 * H // P
    gb = [gp.tile([P, hc, NL], F32, tag=f"gate{g}", bufs=1, name=f"gb{g}")
          for g in range(4)]
    if c_out is None:
        assert cprevT is None
        c_out = gb[1]  # forget-gate buffer is unused for zero-state cells
    for jc in range(nj):
        g = jc // hc
        if cprevT is None and g == 1:
            continue  # forget gate unused with zero initial state
        ps = pg.tile([P, NL], F32, tag="pg", bufs=4, name="ps_gate")
        js = slice(jc * P, (jc + 1) * P)
        for i in range(NE // 2):
            ks = slice(2 * i, 2 * i + 2)
            nc.tensor.matmul(
                ps[:], W_sb[:, ks, js], xT8[:, ks, :],
                start=(i == 0), stop=(U_sb is None and i == NE // 2 - 1),
                perf_mode=DR)
        if U_sb is not None:
            nc.tensor.matmul(
                ps[:], U_sb[:, 0:NH, js], h8[:, 0:NH, :],
                start=False, stop=True, perf_mode=DR)
        func = AF.Tanh if g == 2 else AF.Sigmoid
        nc.scalar.activation(
            out=gb[g][:, jc % hc, :], in_=ps[:], func=func,
            bias=bias_sb[:, jc:jc + 1], scale=1.0 / WSCALE)
    for s in range(hc):
        i_, g_, o_ = gb[0][:, s, :], gb[2][:, s, :], gb[3][:, s, :]
        if cprevT is None:
            nc.vector.tensor_mul(c_out[:, s, :], i_, g_)
        else:
            f_ = gb[1][:, s, :]
            ig = tp.tile([P, NL], F32, tag="ig", bufs=2, name="ig")
            nc.vector.tensor_mul(ig[:], i_, g_)
            nc.vector.tensor_mul(c_out[:, s, :], f_, cprevT[:, s, :])
            nc.vector.tensor_add(c_out[:, s, :], c_out[:, s, :], ig[:])
        tc_ = tp.tile([P, NL], F32, tag="tanhc", bufs=2, name="tanhc")
        nc.scalar.activation(out=tc_[:], in_=c_out[:, s, :], func=AF.Tanh)
        nc.vector.tensor_mul(h_out[:, s, :], o_, tc_[:])
        if h_out8 is not None:
            nc.vector.tensor_copy(out=h_out8[:, s, :],
                                  in_=h_out[:, s, :].bitcast(F32))


def _gather_T(nc, pools, emb, idx_dram, identr, dst8, dst32=None):
    """Gather NL embedding rows, transpose, cast to fp8 (and opt. fp32)."""
    ip, rp, ptp = pools["idx"], pools["raw"], pools["pt"]
    for t in range(NL // P):
        idx_t = ip.tile([P, 1], I32, tag="idx", bufs=4, name="idx_t")
        nc.sync.dma_start(out=idx_t[:], in_=idx_dram[t * P:(t + 1) * P, :])
        raw = rp.tile([P, E], F32R, tag="raw", bufs=4, name="raw")
        nc.gpsimd.indirect_dma_start(
            out=raw[:], out_offset=None, in_=emb[:],
            in_offset=bass.IndirectOffsetOnAxis(ap=idx_t[:, :1], axis=0))
        ptile = ptp.tile([P, NE, P], F32R, tag="pt", bufs=3, name="ptile")
        for et in range(NE):
            nc.tensor.transpose(
                out=ptile[:, et, :], in_=raw[:, et * P:(et + 1) * P],
                identity=identr[:])
        sl = slice(t * P, (t + 1) * P)
        nc.vector.tensor_copy(out=dst8[:, :, sl],
                              in_=ptile[:].bitcast(F32))
        if dst32 is not None:
            nc.scalar.activation(out=dst32[:, :, sl],
                                 in_=ptile[:].bitcast(F32), func=AF.Copy)


def build_program():
    nc = bacc.Bacc("TRN2", target_bir_lowering=False, debug=False,
                   enable_asserts=False, num_devices=NCORES)
    dram = lambda name, shape, dt=F32, kind="ExternalInput": \
        nc.dram_tensor(name, shape, dt, kind=kind).ap()

    emb = dram("emb", [V, E])
    idx_x = dram("idx_x", [NL, 1], I32)
    idx_e0 = dram("idx_e0", [NL, 1], I32)
    idx_e1 = dram("idx_e1", [NL, 1], I32)
    # weights pre-laid-out on host as fp8 lhsT tiles [p, kt, j], x16
    wgf = dram("wgf", [P, NE, 4 * G], F8)
    wgr = dram("wgr", [P, NE, 4 * G], F8)
    ugf = dram("ugf", [P, NH, 4 * G], F8)
    ugr = dram("ugr", [P, NH, 4 * G], F8)
    wf = dram("wf", [P, NE, 4 * E], F8)
    bgf = dram("bgf", [P, 8])
    bgr = dram("bgr", [P, 8])
    bf = dram("bf", [P, 16])
    out_dot = dram("out_dot", [2, E], kind="ExternalOutput")
    out_sr = dram("out_sr", [2, E], kind="ExternalOutput")
    out_sg = dram("out_sg", [2, E], kind="ExternalOutput")

    with tile.TileContext(nc) as tc:
        _emit(tc, locals())
    nc.compile()
    return nc


def _emit(tc, T):
    nc = tc.nc
    rg = [list(range(NCORES))]
    from contextlib import ExitStack
    ctx = ExitStack()
    with ctx:
        glob = ctx.enter_context(tc.tile_pool(name="glob", bufs=1))
        dramp = ctx.enter_context(tc.tile_pool(name="dramp", bufs=1, space="DRAM"))

        # tiny warmup collective: absorbs the device barrier + first-
        # collective ncfw warmup (~45us) while local compute proceeds.
        # No producers (payload is scratch) so the trigger is ready at t=0.
        warm_src = dramp.tile([1, 256], F8)
        warm_dst = dramp.tile([NCORES, 256], F8, addr_space="Shared")
        with tc.high_priority():
            nc.gpsimd.collective_compute(
                "AllGather", ALU.bypass, replica_groups=rg,
                ins=[warm_src[:].opt()], outs=[warm_dst[:].opt()])

        ident = glob.tile([P, P], F32)
        make_identity(nc, ident)
        identr = glob.tile([P, P], F32R)
        nc.vector.tensor_copy(out=identr[:], in_=ident[:])

        # flat collective bounce buffers (fp8): descriptor-row count, not
        # bytes, dominates collective cost - declare 1 row, view for DMA
        ag1_src = dramp.tile([1, E * NL], F8)
        ag1_dst = dramp.tile([NCORES, E * NL], F8, addr_space="Shared")
        ag2_src = dramp.tile([1, 2 * NL * E], F8)
        ag2_dst = dramp.tile([NCORES, 2 * NL * E], F8, addr_space="Shared")
        ag1w = ag1_src.rearrange("a (e n) -> (a e) n", n=NL)       # [E, NL]
        ag2w = ag2_src.rearrange("a (m e) -> (a m) e", e=E)        # [2NL, E]

        # long-lived activations
        xT8 = glob.tile([P, NE, NL], F8)
        xT = glob.tile([P, NE, NL], F32)
        e0T8 = glob.tile([P, NE, NL], F8)
        e1T8 = glob.tile([P, NE, NL], F8)
        g0T = glob.tile([P, NE, NL], F32R)
        g1T = glob.tile([P, NE, NL], F32R)
        h08 = glob.tile([P, NH, NL], F8)   # hf0 fp8 (U input of fwd step 2)
        h18 = glob.tile([P, NH, NL], F8)   # hr1 fp8 (U input of rev step 2)
        dgT8 = glob.tile([P, NE, NL], F8)
        A0T = glob.tile([P, NMB, NL], F8)
        A1T = glob.tile([P, NMB, NL], F8)

        with tc.tile_pool(name="wpool", bufs=1) as wp, \
             tc.tile_pool(name="acts", bufs=1) as ap_, \
             tc.tile_pool(name="gates", bufs=1) as gp, \
             tc.tile_pool(name="tmp", bufs=1) as tp, \
             tc.tile_pool(name="idx", bufs=1) as ip, \
             tc.tile_pool(name="raw", bufs=1) as rp, \
             tc.tile_pool(name="pg", bufs=1, space="PSUM") as pgp, \
             tc.tile_pool(name="pt", bufs=1, space="PSUM") as ptp:
            pools = {"pg": pgp, "gates": gp, "tmp": tp, "idx": ip,
                     "raw": rp, "pt": ptp}

            # ---- phase A: x gather -> f-cell -> fh -> AG1 (early) ----
            _gather_T(nc, pools, T["emb"], T["idx_x"], identr, xT8, xT)
            wf_sb = wp.tile([P, NE, 4 * E], F8)
            nc.sync.dma_start(out=wf_sb[:], in_=T["wf"][:])
            bf_sb = wp.tile([P, 16], F32)
            nc.sync.dma_start(out=bf_sb[:], in_=T["bf"][:])

            fhT = ap_.tile([P, NE, NL], F32, name="fhT")
            _lstm_cell8(nc, pools, E, xT8, wf_sb, None, None, bf_sb, fhT,
                        None)
            fhT8 = ap_.tile([P, NE, NL], F8, name="fhT8")
            for et in range(NE):
                nc.vector.tensor_add(fhT[:, et, :], fhT[:, et, :],
                                     xT[:, et, :])
                nc.vector.tensor_copy(out=fhT8[:, et, :], in_=fhT[:, et, :])
                nc.sync.dma_start(
                    out=ag1w[et * P:(et + 1) * P, :], in_=fhT8[:, et, :])
            nc.gpsimd.collective_compute(
                "AllGather", ALU.bypass, replica_groups=rg,
                ins=[ag1_src[:].opt()], outs=[ag1_dst[:].opt()])

            # ---- phase B: e gathers -> g cells -> transposes -> AG2 ----
            _gather_T(nc, pools, T["emb"], T["idx_e0"], identr, e0T8)
            _gather_T(nc, pools, T["emb"], T["idx_e1"], identr, e1T8)
            w_sb = {}
            for nm in ("wgf", "wgr", "ugf", "ugr"):
                shp = [P, NE, 4 * G] if nm[0] == "w" else [P, NH, 4 * G]
                w_sb[nm] = wp.tile(shp, F8, name=nm + "_sb")
                nc.sync.dma_start(out=w_sb[nm][:], in_=T[nm][:])
            for nm in ("bgf", "bgr"):
                w_sb[nm] = wp.tile([P, 8], F32, name=nm + "_sb")
                nc.sync.dma_start(out=w_sb[nm][:], in_=T[nm][:])

            cfT = ap_.tile([P, NH, NL], F32, name="cfT")
            crT = ap_.tile([P, NH, NL], F32, name="crT")
            c2T = ap_.tile([P, NH, NL], F32, name="c2T")
            c3T = ap_.tile([P, NH, NL], F32, name="c3T")
            hf0 = g0T[:, 0:NH, :]
            hf1 = g1T[:, 0:NH, :]
            hr1 = g1T[:, NH:NE, :]
            hr0 = g0T[:, NH:NE, :]
            _lstm_cell8(nc, pools, G, e0T8, w_sb["wgf"], None, None,
                        w_sb["bgf"], hf0, cfT, h_out8=h08)
            _lstm_cell8(nc, pools, G, e1T8, w_sb["wgr"], None, None,
                        w_sb["bgr"], hr1, crT, h_out8=h18)
            _lstm_cell8(nc, pools, G, e1T8, w_sb["wgf"], w_sb["ugf"], h08,
                        w_sb["bgf"], hf1, c2T, cprevT=cfT)
            _lstm_cell8(nc, pools, G, e0T8, w_sb["wgr"], w_sb["ugr"], h18,
                        w_sb["bgr"], hr0, c3T, cprevT=crT)

            # dg = g0 - g1, scaled x8 into fp8 (unscaled in the D sigmoid)
            for et in range(NE):
                dgf = tp.tile([P, NL], F32, tag="dgf", bufs=2, name="dgf")
                nc.vector.tensor_sub(dgf[:], g0T[:, et, :].bitcast(F32),
                                     g1T[:, et, :].bitcast(F32))
                nc.vector.tensor_scalar_mul(
                    out=dgT8[:, et, :], in0=dgf[:], scalar1=DGSCALE)

            # transpose g0/g1 into ag2_src (fp8) and fire AG2.
            # high_priority: the stile DMAs and the trigger must win queue
            # slots over AG1-gated loads and reduction outputs.
            with tc.high_priority():
                for srcT, row0 in ((g0T, 0), (g1T, NL)):
                    for nt in range(NL // P):
                        ptile = ptp.tile([P, NE, P], F32R, tag="pt", bufs=3,
                                         name="ptg")
                        for et in range(NE):
                            nc.tensor.transpose(
                                out=ptile[:, et, :],
                                in_=srcT[:, et, nt * P:(nt + 1) * P],
                                identity=identr[:])
                        stile = tp.tile([P, E], F8, tag="tps", bufs=8,
                                        name="stile")
                        nc.vector.tensor_copy(out=stile[:],
                                              in_=ptile[:].bitcast(F32))
                        nc.sync.dma_start(
                            out=ag2w[row0 + nt * P:row0 + (nt + 1) * P, :],
                            in_=stile[:])
                nc.gpsimd.collective_compute(
                    "AllGather", ALU.bypass, replica_groups=rg,
                    ins=[ag2_src[:].opt()], outs=[ag2_dst[:].opt()])


        # ---- phase C: D.T blocks; A0 = sig(D/8), A1 = 1 - A0 (fp8) ----
        with tc.tile_pool(name="fhk", bufs=1) as fkp, \
             tc.tile_pool(name="gS", bufs=1) as gsp, \
             tc.tile_pool(name="fin", bufs=1) as fin, \
             tc.tile_pool(name="pd", bufs=1, space="PSUM") as pdp, \
             tc.tile_pool(name="pr", bufs=1, space="PSUM") as prp:
            for k in range(NCORES):
                fhk = fkp.tile([P, NE, NL], F8, tag="fhk", bufs=8, name="fhk")
                nc.sync.dma_start(
                    out=fhk[:],
                    in_=ag1_dst[k:k + 1, :].rearrange(
                        "k (et p n) -> (k p) et n", p=P, n=NL))
                for c in range(NL // P):
                    mb = k * (NL // P) + c
                    pd = pdp.tile([P, NL], F32, tag="pd", bufs=4, name="pd")
                    cs = slice(c * P, (c + 1) * P)
                    for i in range(NE // 2):
                        ks = slice(2 * i, 2 * i + 2)
                        nc.tensor.matmul(
                            pd[:], fhk[:, ks, cs], dgT8[:, ks, :],
                            start=(i == 0), stop=(i == NE // 2 - 1),
                            perf_mode=DR)
                    nc.scalar.activation(
                        out=A0T[:, mb, :], in_=pd[:], func=AF.Sigmoid,
                        scale=1.0 / DGSCALE)
                for c2 in range(0, NL // P, 2):
                    mb = k * (NL // P) + c2
                    nc.vector.tensor_scalar(
                        out=A1T[:, mb:mb + 2, :], in0=A0T[:, mb:mb + 2, :],
                        scalar1=-1.0, scalar2=1.0, op0=ALU.mult, op1=ALU.add)

            # ---- phase D: r via fp8 DoubleRow, g cached in SBUF ----
            g0S = gsp.tile([P, NMB, E], F8, name="g0S")
            g1S = gsp.tile([P, NMB, E], F8, name="g1S")
            nb = NL // P
            for gS, off in ((g0S, 0), (g1S, NL * E)):
                for k in range(NCORES):
                    nc.sync.dma_start(
                        out=gS[:, k * nb:(k + 1) * nb, :],
                        in_=ag2_dst[k:k + 1, off:off + NL * E].rearrange(
                            "k (c p e) -> (k p) c e", p=P, e=E))

            # sg_b = sum_n g_b^2 (local, fp32) - emitted late so its
            # scalar ops and tiny DMAs sit behind the D-phase in queue order
            for b, gT in ((0, g0T), (1, g1T)):
                for et in range(NE):
                    scr3 = fin.tile([P, NL], F32, tag="scr3", bufs=4,
                                    name="scr3")
                    asg = fin.tile([P, 1], F32, tag="asg", bufs=8, name="asg")
                    nc.scalar.activation(out=scr3[:],
                                         in_=gT[:, et, :].bitcast(F32),
                                         func=AF.Square, accum_out=asg[:])
                    nc.sync.dma_start(out=T["out_sg"][b, et * P:(et + 1) * P],
                                      in_=asg[:])

            for et in range(NE):
                es = slice(et * P, (et + 1) * P)
                r0 = prp.tile([P, NL], F32, tag="r0", bufs=2, name="r0")
                r1 = prp.tile([P, NL], F32, tag="r1", bufs=2, name="r1")
                for i in range(NMB // 2):
                    ms = slice(2 * i, 2 * i + 2)
                    nc.tensor.matmul(
                        r0[:], g0S[:, ms, es], A0T[:, ms, :],
                        start=(i == 0), stop=(i == NMB // 2 - 1),
                        perf_mode=DR)
                    nc.tensor.matmul(
                        r1[:], g1S[:, ms, es], A1T[:, ms, :],
                        start=(i == 0), stop=(i == NMB // 2 - 1),
                        perf_mode=DR)
                # sr and dot reductions for this e-chunk (fp32)
                for b, (rr, gT) in enumerate(((r0, g0T), (r1, g1T))):
                    scr2 = fin.tile([P, NL], F32, tag="scr2", bufs=4,
                                    name="scr2")
                    asr = fin.tile([P, 1], F32, tag="asr", bufs=4,
                                   name="asr")
                    nc.scalar.activation(out=scr2[:], in_=rr[:],
                                         func=AF.Square, accum_out=asr[:])
                    nc.sync.dma_start(
                        out=T["out_sr"][b, et * P:(et + 1) * P], in_=asr[:])
                    scr = fin.tile([P, NL], F32, tag="scr", bufs=4,
                                   name="scr")
                    adot = fin.tile([P, 1], F32, tag="adot", bufs=4,
                                    name="adot")
                    nc.vector.tensor_mul(scr[:], rr[:],
                                         gT[:, et, :].bitcast(F32))
                    nc.vector.reduce_sum(out=adot[:], in_=scr[:],
                                         axis=mybir.AxisListType.X)
                    nc.sync.dma_start(
                        out=T["out_dot"][b, et * P:(et + 1) * P],
                        in_=adot[:])


_PROGRAM = None


def _get_program():
    global _PROGRAM
    if _PROGRAM is None:
        _PROGRAM = build_program()
    return _PROGRAM


def _prep_w8(w, scale=WSCALE):
    """(4H, E_in) torch-layout weight -> fp8 lhsT tiles [p, kt, 4H], scaled."""
    wt = np.asarray(w, np.float32).T * scale  # (E_in, 4H)
    e_in, fourh = wt.shape
    arr = np.ascontiguousarray(
        wt.reshape(e_in // P, P, fourh).transpose(1, 0, 2))
    arr = np.clip(arr, -240.0, 240.0)
    return arr.astype(mybir.dt.np(F8))


def _prep_b(b1, b2):
    s = (np.asarray(b1, np.float32) + np.asarray(b2, np.float32))
    return np.ascontiguousarray(s.reshape(-1, P).T)


def run_device(inputs, trace=False):
    """Shard inputs, run the 8-core SPMD program, return bass results."""
    nc = _get_program()
    emb = np.ascontiguousarray(np.asarray(inputs["embedding"], np.float32))
    iq = np.asarray(inputs["input"]).astype(np.int32).reshape(N, 1)
    ie = np.asarray(inputs["set_inputs"]).astype(np.int32)
    shared = {
        "emb": emb,
        "wgf": _prep_w8(inputs["wih_gf"]), "wgr": _prep_w8(inputs["wih_gr"]),
        "ugf": _prep_w8(inputs["whh_gf"]), "ugr": _prep_w8(inputs["whh_gr"]),
        "wf": _prep_w8(inputs["wih_f"]),
        "bgf": _prep_b(inputs["bih_gf"], inputs["bhh_gf"]),
        "bgr": _prep_b(inputs["bih_gr"], inputs["bhh_gr"]),
        "bf": _prep_b(inputs["bih_f"], inputs["bhh_f"]),
    }
    in_maps = []
    for k in range(NCORES):
        sl = slice(k * NL, (k + 1) * NL)
        m = dict(shared)
        m["idx_x"] = np.ascontiguousarray(iq[sl])
        m["idx_e0"] = np.ascontiguousarray(ie[0, sl].reshape(NL, 1))
        m["idx_e1"] = np.ascontiguousarray(ie[1, sl].reshape(NL, 1))
        in_maps.append(m)
    res = bass_utils.run_bass_kernel_spmd(
        nc, in_maps, core_ids=list(range(NCORES)), trace=trace)
    return res


def kernel(**inputs):
    res = run_device(inputs)
    return host_tail(res, inputs)


def host_tail(res, inputs):
    dot = np.zeros((2, E), np.float64)
    sr = np.zeros((2, E), np.float64)
    sg = np.zeros((2, E), np.float64)
    for r in res.results:
        dot += r["out_dot"]
        sr += r["out_sr"]
        sg += r["out_sg"]
    nr = np.maximum(np.sqrt(sr), EPS)
    ng = np.maximum(np.sqrt(sg), EPS)
    cos = dot / (nr * ng)                        # (2, E)
    kern = cos / np.exp(cos).sum()
    w_out = np.asarray(inputs["w_out"], np.float64)
    b_out = np.asarray(inputs["b_out"], np.float64)
    k2 = kern @ w_out.T + b_out                  # (2, R)
    s = k2.sum(axis=1)                           # (2,)
    labels = np.asarray(inputs["set_labels"], np.float64)
    o = s[0] * labels[0] + s[1] * labels[1]      # (R,)
    o = np.exp(o - o.max())
    o /= o.sum()
    return o.astype(np.float32)
